# revision 57
# baseline (speedup 1.0000x reference)
"""GAT (2-layer dense-graph attention over 4096 nodes) as a Trainium2
Bass/Tile SPMD kernel across 8 NeuronCores.

v3 design:
- Layer 0 DST-sharded (512 destination rows/core). Layer 1 ALSO
  dst-sharded: the elu'd layer-0 output (contp, [64,512] bf16/core) is
  AllGathered and every core computes its own 512 rows of layer 1
  locally in f32 PSUM (no ReduceScatter, no bf16 partial-sum loss).
  AG1 (heads 0-3) fires halfway through layer 0 and is hidden under
  pass 2; only AG2's latency is exposed (filled with AG1-dependent
  layer-1 prep matmuls, local-rank work and keep-warm matmuls).
- Layer-0 attention: 2 passes x 4 heads. Head pairs share a PSUM bank
  (odd head at tile_position col 32); the bank is DVE-memset to zero
  and every matmul uses start=False, which is correct regardless of
  stale has_written state. Inner loop is jt-major so the even/odd-head
  matmuls of a pair hit different 32-col groups back-to-back and can
  overlap in the PE sub-arrays.
- E-tile = e^{d_j} * max(a_i, e^{-0.8 d_j}) with e^{d_j} folded into
  the stationary ([e^d | h'*e^d], 9 cols, ones-free): ONE single-
  scalar DVE tensor_scalar(max) per (h,jt) tile for 3 heads/pass; the
  4th head runs on ScalarE as relu(e^{-0.8 d_j} - a_i) (per-partition
  bias AP) plus an exact rank-1 correction a_i * (sum_j e^{d_j} h'_j)
  accumulated via tiny K=1 matmuls, so DVE and ScalarE generate
  E-tiles concurrently.
- Normalize+elu is transpose-first: all per-dst math runs dst-major on
  128 lanes; one bf16 transpose per 128-dst chunk writes 32-aligned
  feature-major contp blocks.
- Output path: transpose, then per-partition reciprocal+scale.
"""

import numpy as np
import ml_dtypes

import concourse.bacc as bacc
import concourse.mybir as mybir
import concourse.tile as tile
from concourse import masks
from concourse.bass_utils import run_bass_kernel_spmd

F32 = mybir.dt.float32
BF16 = mybir.dt.bfloat16
AF = mybir.ActivationFunctionType
OP = mybir.AluOpType
N = 4096
NCORES = 8
RPC = N // NCORES          # rows per core = 512
NJT = N // 128             # 32 j-tiles of 128 source rows
BN_EPS = 1e-5

# head offloaded to ScalarE per pass (pass0: heads 0-3, pass1: 4-7);
# must be even (pair-bank col offset 0)
SCE_HEADS = (0, 4)

_CACHE = {}


def _build():
    nc = bacc.Bacc("TRN2", target_bir_lowering=False, debug=False,
                   num_devices=NCORES)

    xTb_d = nc.dram_tensor("xTb", [33, N], BF16, kind="ExternalInput")
    xsTb_d = nc.dram_tensor("xsTb", [33, RPC], BF16, kind="ExternalInput")
    w0allb_d = nc.dram_tensor("w0allb", [33, 72], BF16, kind="ExternalInput")
    w0sb_d = nc.dram_tensor("w0sb", [33, 8], BF16, kind="ExternalInput")
    w1ea_d = nc.dram_tensor("w1ea", [32, 34], BF16, kind="ExternalInput")
    w1el_d = nc.dram_tensor("w1el", [33, 34], BF16, kind="ExternalInput")
    w1s1_d = nc.dram_tensor("w1s1", [65, 1], BF16, kind="ExternalInput")
    sela_d = nc.dram_tensor("sela", [8, 8 * 128], BF16, kind="ExternalInput")
    onesN_d = nc.dram_tensor("onesN", [1, N], BF16, kind="ExternalInput")
    out_d = nc.dram_tensor("out", [RPC, 32], F32, kind="ExternalOutput")

    with tile.TileContext(nc) as tc:
        with (
            tc.tile_pool(name="const", bufs=1) as const,
            tc.tile_pool(name="per", bufs=1) as per,
            tc.tile_pool(name="dram", bufs=1, space="DRAM") as dram,
        ):
            # ---------- dram intermediates ----------
            dum_i = dram.tile([1, 8], F32, name="dum_i", tag="dum_i")
            dum_o = dram.tile([8, 8], F32, name="dum_o", tag="dum_o")
            ag1i = dram.tile([32, RPC], BF16, name="ag1i", tag="ag1i")
            ag1o = dram.tile([NCORES, 32, RPC], BF16, name="ag1o", tag="ag1o")
            ag2i = dram.tile([32, RPC], BF16, name="ag2i", tag="ag2i")
            ag2o = dram.tile([NCORES, 32, RPC], BF16, name="ag2o", tag="ag2o")

            # dummy collective first: absorbs device barrier + CC warmup
            nc.gpsimd.collective_compute(
                "AllGather", OP.bypass,
                replica_groups=[list(range(NCORES))],
                ins=[dum_i.opt()], outs=[dum_o.opt()])

            # ---------- persistent sbuf ----------
            xTb = per.tile([33, N], BF16)
            xsTb = per.tile([33, RPC], BF16)
            atile = per.tile([128, 8, RPC], BF16)      # e^{0.8 s0} bcast
            hpa0 = per.tile([128, NJT, 8, 9], BF16)    # [e^d | h'0*e^d]
            d0raw = per.tile([128, NJT * 8], F32)      # d0 scores j-major
            b0f = per.tile([128, NJT * 8], F32)        # e^{d0}
            g0 = per.tile([128, NJT * 8], F32)         # e^{-0.8 d0}
            uT = [per.tile([1, 9], BF16, name=f"uT{k}", tag=f"uT{k}")
                  for k in range(2)]
            sU = per.tile([9, 2], F32)
            contp = per.tile([65, RPC], BF16)          # elu rows + ones row
            cAll1 = per.tile([32, N], BF16)            # gathered elu 0:32
            cAll2 = per.tile([33, N], BF16)            # elu 32:64 + ones
            h1acc = per.tile([128, NJT, 34], F32)      # w1[0:32] partials
            hpa1 = per.tile([128, NJT, 33], BF16)      # [h'1*e^d1 | e^d1]
            d1raw = per.tile([128, NJT], F32)
            b1 = per.tile([128, NJT], F32)             # e^{d1}
            g01 = per.tile([128, NJT], F32)            # e^{-0.8 d1}
            a1b = per.tile([1, RPC], BF16)
            atile1 = per.tile([128, RPC], BF16)
            os1 = per.tile([33, RPC], F32)             # L1 numerator evac

            # ---------- consts (cheap memsets first for warmup) ----------
            wsrc = const.tile([128, 512], BF16)
            nc.vector.memset(wsrc[:], 0.5)
            wlhs = const.tile([128, 128], BF16)
            nc.vector.memset(wlhs[:], 0.25)
            ones128 = const.tile([1, 128], BF16)
            nc.vector.memset(ones128[:], 1.0)
            onesc = const.tile([128, 1], BF16)
            nc.vector.memset(onesc[:], 1.0)
            ident = const.tile([128, 128], F32)
            masks.make_identity(nc, ident[:])
            nc.sync.dma_start(xsTb[:], xsTb_d[:])
            sela = const.tile([8, 8 * 128], BF16)
            nc.sync.dma_start(sela[:], sela_d[:])
            w0allb = const.tile([33, 72], BF16)
            nc.sync.dma_start(w0allb[:], w0allb_d[:])
            w0sb = const.tile([33, 8], BF16)
            nc.sync.dma_start(w0sb[:], w0sb_d[:])
            w1ea = const.tile([32, 34], BF16)
            nc.sync.dma_start(w1ea[:], w1ea_d[:])
            w1el = const.tile([33, 34], BF16)
            nc.sync.dma_start(w1el[:], w1el_d[:])
            w1s1 = const.tile([65, 1], BF16)
            nc.sync.dma_start(w1s1[:], w1s1_d[:])
            nc.sync.dma_start(contp[64:65, :], onesN_d[0:1, 0:RPC])
            nc.sync.dma_start(cAll2[32:33, :], onesN_d[:])
            for cc in range(4):
                nc.sync.dma_start(xTb[:, cc * 1024:(cc + 1) * 1024],
                                  xTb_d[:, cc * 1024:(cc + 1) * 1024])

            # ---------------- Phase 0: warmup ----------------
            with tc.tile_pool(name="wupp", bufs=1, space="PSUM") as wupp:
                wps = wupp.tile([128, RPC], F32, tag="wps")
                for r in range(10):
                    nc.tensor.matmul(wps[:], wlhs[:], wsrc[:],
                                     start=(r == 0), stop=(r == 9))

            # ---------------- Phase 1: s0 -> atile ----------------
            with (
                tc.tile_pool(name="p1s", bufs=2) as p1s,
                tc.tile_pool(name="p1p", bufs=2, space="PSUM") as p1p,
            ):
                ps0 = p1p.tile([8, RPC], F32, tag="pa")
                nc.tensor.matmul(ps0[:], w0sb[:], xsTb[:])
                a0row = p1s.tile([8, RPC], BF16, tag="a0row")
                nc.scalar.activation(a0row[:], ps0[:], AF.Exp, scale=0.8)
                for h in range(8):
                    pa = p1p.tile([128, RPC], F32, tag="pa")
                    nc.tensor.matmul(pa[:], sela[:, h * 128:(h + 1) * 128],
                                     a0row[:])
                    if h % 2 == 0:
                        nc.vector.tensor_copy(atile[:, h, :], pa[:])
                    else:
                        nc.scalar.copy(atile[:, h, :], pa[:])

            # -------- Layer-0 attention: two passes of 4 heads --------
            def prep_group(g):
                """h'0/d0 for j-tiles 8g..8g+8 -> hpa0 (e^d folded)."""
                gs = slice(g * 64, (g + 1) * 64)
                gj = slice(g * 8, (g + 1) * 8)
                pts = []
                for half in range(2):
                    pt = ptp.tile([128, 4, 72], F32, tag="ptile")
                    pts.append(pt)
                    for q in range(4):
                        jt = g * 8 + half * 4 + q
                        nc.tensor.matmul(
                            pt[:, q, :],
                            xTb[:, jt * 128:(jt + 1) * 128],
                            w0allb[:])
                        nc.vector.tensor_copy(
                            d0raw[:, jt * 8:(jt + 1) * 8],
                            pt[:, q, 64:72])
                nc.scalar.activation(g0[:, gs], d0raw[:, gs], AF.Exp,
                                     scale=-0.8)
                nc.scalar.activation(b0f[:, gs], d0raw[:, gs], AF.Exp)
                nc.scalar.activation(
                    hpa0[:, gj, :, 0], d0raw[:, gs].rearrange(
                        "p (a b) -> p a b", b=8), AF.Exp)
                for half in range(2):
                    for q in range(4):
                        jt = g * 8 + half * 4 + q
                        bc = b0f[:, jt * 8:(jt + 1) * 8]
                        nc.vector.tensor_tensor(
                            hpa0[:, jt, :, 1:9],
                            pts[half][:, q, 0:64].rearrange(
                                "p (h o) -> p h o", h=8),
                            bc.unsqueeze(2).broadcast_to([128, 8, 8]),
                            op=OP.mult)

            def run_pass(p, pgp, psu, prep):
                heads = list(range(4 * p, 4 * p + 4))
                sce_h = SCE_HEADS[p]
                psU = psu.tile([9, 1], F32, tag="psU")
                pgs = {}
                for h in heads:
                    # one PSUM bank per head: adjacent matmuls alternate
                    # 32-col groups (concurrency) but never share a bank
                    pgs[h] = pgp.tile([41, RPC], F32,
                                      name=f"pg_{p}_{h}",
                                      tag=f"pg{h % 4}")
                    if h % 2 == 0:
                        nc.vector.memset(pgs[h][0:9, :], 0.0)
                    else:
                        nc.vector.memset(pgs[h][32:41, :], 0.0)
                if prep:  # stay one group ahead so ScE exps clear the
                    prep_group(0)  # queue before the group's relu tiles
                    prep_group(1)
                for g in range(4):
                    if prep and 2 <= g + 1 <= 3:
                        prep_group(g + 1)
                    for j8 in range(8):
                        jt = g * 8 + j8
                        es = {}
                        for h in heads:
                            col = jt * 8 + h
                            e = epool.tile([128, RPC], BF16, tag="e")
                            es[h] = e
                            if h == sce_h:
                                nc.scalar.activation(
                                    e[:], atile[:, h, :], AF.Relu,
                                    bias=g0[:, col:col + 1],
                                    scale=-1.0)
                            elif h == sce_h + 2 and j8 in (3, 7):
                                # idle GPSIMD as a third E-tile engine
                                nc.gpsimd.tensor_scalar(
                                    e[:], atile[:, h, :],
                                    g0[:, col:col + 1], None,
                                    op0=OP.max)
                            else:
                                nc.vector.tensor_scalar(
                                    e[:], atile[:, h, :],
                                    g0[:, col:col + 1], None,
                                    op0=OP.max)
                        for h in heads:
                            hv = pgs[h][0:9, :] if h % 2 == 0 \
                                else pgs[h][32:41, :]
                            tp = (0, 0) if h % 2 == 0 else (0, 32)
                            nc.tensor.matmul(
                                hv, hpa0[:, jt, h, :], es[h][:],
                                start=False,
                                stop=(jt == NJT - 1 and h != sce_h),
                                skip_group_check=True,
                                tile_position=tp)
                            if h == sce_h:
                                nc.tensor.matmul(
                                    psU[:], hpa0[:, jt, h, :],
                                    onesc[:],
                                    start=(jt == 0),
                                    stop=(jt == NJT - 1))
                # rank-1 correction for the ScalarE head (even => col 0)
                nc.vector.tensor_copy(sU[:, p:p + 1], psU[:])
                put = psu.tile([1, 9], F32, tag="psUT")
                nc.tensor.matmul(put[:], sU[:, p:p + 1], ident[0:9, 0:9],
                                 is_transpose=True)
                nc.vector.tensor_copy(uT[p][:], put[:])
                nc.tensor.matmul(pgs[sce_h][0:9, :], uT[p][:],
                                 atile[0:1, sce_h, :],
                                 start=False, stop=True,
                                 skip_group_check=True)
                return pgs

            def finalize_pass(p, pgs, agi, finp, sbfp):
                """normalize+elu for the pass's 4 heads -> contp rows
                32p:32p+32, transpose-first (all dst-major on 128
                lanes), then bf16 transpose back per 128-dst chunk."""
                sbA = sbfp.tile([41, RPC], F32, tag="sbf")
                nc.vector.tensor_copy(sbA[0:9, :], pgs[4 * p][0:9, :])
                nc.scalar.copy(sbA[32:41, :], pgs[4 * p + 1][32:41, :])
                sbB = sbfp.tile([41, RPC], F32, tag="sbf")
                nc.vector.tensor_copy(sbB[0:9, :], pgs[4 * p + 2][0:9, :])
                nc.scalar.copy(sbB[32:41, :], pgs[4 * p + 3][32:41, :])
                for q in range(4):
                    nm0 = sbfp.tile([128, 32], F32, tag="nm0")
                    for c2, sb in ((0, sbA), (1, sbB)):
                        ptx = finp.tile([128, 41], F32, tag="ptx")
                        nc.tensor.matmul(ptx[:],
                                         sb[:, q * 128:(q + 1) * 128],
                                         ident[0:41, 0:41],
                                         is_transpose=True)
                        rq2 = sbfp.tile([128, 2], F32, tag="rq2")
                        nc.vector.reciprocal_approx_fast(
                            rq2[:, 0:1], ptx[:, 0:1])
                        nc.vector.reciprocal_approx_fast(
                            rq2[:, 1:2], ptx[:, 32:33])
                        nc.vector.tensor_scalar(
                            nm0[:, 16 * c2:16 * c2 + 8], ptx[:, 1:9],
                            rq2[:, 0:1], None, op0=OP.mult)
                        nc.vector.tensor_scalar(
                            nm0[:, 16 * c2 + 8:16 * c2 + 16],
                            ptx[:, 33:41],
                            rq2[:, 1:2], None, op0=OP.mult)
                    # contp carries elu+1 = max(x,0) + min(e^x,1); the
                    # -1 is folded into the layer-1 beta/d/s rows
                    texq = sbfp.tile([128, 32], F32, tag="tex")
                    nc.scalar.activation(texq[:], nm0[:], AF.Exp)
                    t1q = sbfp.tile([128, 32], F32, tag="tex")
                    nc.vector.tensor_scalar(t1q[:], texq[:], 1.0, None,
                                            op0=OP.min)
                    eluq = sbfp.tile([128, 32], F32, tag="eluq")
                    nc.vector.scalar_tensor_tensor(
                        eluq[:], nm0[:], 0.0, t1q[:],
                        op0=OP.max, op1=OP.add)
                    ptb = finp.tile([32, 128], F32, tag="ptb")
                    nc.tensor.matmul(ptb[:], eluq[:],
                                     ident[0:128, 0:128],
                                     is_transpose=True)
                    nc.vector.tensor_copy(
                        contp[32 * p:32 * p + 32,
                              q * 128:(q + 1) * 128], ptb[:])
                nc.sync.dma_start(agi[:],
                                  contp[32 * p:32 * p + 32, :])

            with (
                tc.tile_pool(name="epool", bufs=8) as epool,
                tc.tile_pool(name="sbfp", bufs=2) as sbfp,
            ):
                with tc.tile_pool(name="pgp", bufs=1, space="PSUM") as pgp:
                    with (
                        tc.tile_pool(name="ptp", bufs=2,
                                     space="PSUM") as ptp,
                        tc.tile_pool(name="psu", bufs=1,
                                     space="PSUM") as psu,
                    ):
                        pgs1 = run_pass(0, pgp, psu, prep=True)
                    with (
                        tc.tile_pool(name="fin", bufs=1,
                                     space="PSUM") as finp,
                        tc.tile_pool(name="psu2", bufs=1,
                                     space="PSUM") as psu2,
                    ):
                        finalize_pass(0, pgs1, ag1i, finp, sbfp)
                        nc.gpsimd.collective_compute(
                            "AllGather", OP.bypass,
                            replica_groups=[list(range(NCORES))],
                            ins=[ag1i.opt()], outs=[ag1o.opt()])
                        for r in range(NCORES):
                            nc.sync.dma_start(
                                cAll1[:, r * RPC:(r + 1) * RPC],
                                ag1o[r, :, :])
                        pgs2 = run_pass(1, pgp, psu2, prep=False)
                    with (
                        tc.tile_pool(name="fin2", bufs=1,
                                     space="PSUM") as finp2,
                        tc.tile_pool(name="p1ap", bufs=1,
                                     space="PSUM") as p1ap,
                        tc.tile_pool(name="tp2a", bufs=1,
                                     space="PSUM") as tp2a,
                    ):
                        # AG1-dependent layer-1 prep (runs while pass-2
                        # finalize uses DVE/ScE): w1[0:32] contribution
                        for q in range(NJT // 4):
                            pa1 = p1ap.tile([128, 4, 34], F32, tag="p1x")
                            for k in range(4):
                                jt = 4 * q + k
                                nc.tensor.matmul(
                                    pa1[:, k, :],
                                    cAll1[:, jt * 128:(jt + 1) * 128],
                                    w1ea[:])
                            nc.vector.tensor_copy(
                                h1acc[:, 4 * q:4 * q + 4, :], pa1[:])
                        finalize_pass(1, pgs2, ag2i, finp2, sbfp)
                        # own-rank L1 pieces + a1 broadcast (local data;
                        # must come AFTER finalize writes contp 32:64)
                        ps1r = tp2a.tile([128, RPC], F32, tag="pbig")
                        nc.tensor.matmul(ps1r[0:1, :], w1s1[:], contp[:])
                        nc.scalar.activation(a1b[:], ps1r[0:1, :], AF.Exp,
                                             scale=0.8)
                        pa1b = tp2a.tile([128, RPC], F32, tag="pbig")
                        nc.tensor.matmul(pa1b[:], ones128[:], a1b[:])
                        nc.vector.tensor_copy(atile1[:], pa1b[:])

            nc.gpsimd.collective_compute(
                "AllGather", OP.bypass,
                replica_groups=[list(range(NCORES))],
                ins=[ag2i.opt()], outs=[ag2o.opt()])

            # ---------------- Layer 1 ----------------
            with (
                tc.tile_pool(name="p1bp", bufs=2, space="PSUM") as p1bp,
                tc.tile_pool(name="agg1", bufs=1, space="PSUM") as agg1,
                tc.tile_pool(name="tp2", bufs=2, space="PSUM") as tp2,
                tc.tile_pool(name="hsp", bufs=2) as hsp,
                tc.tile_pool(name="e1pool", bufs=4) as e1pool,
                tc.tile_pool(name="otp", bufs=2) as otp,
            ):
                for r in range(NCORES):
                    nc.sync.dma_start(cAll2[0:32, r * RPC:(r + 1) * RPC],
                                      ag2o[r, :, :])
                # keep PE warm across the AG2 wait (also spans the gap
                # so the layer-1 stream starts at full clock)
                wps2 = tp2.tile([128, RPC], F32, tag="pbig")
                for r in range(45):
                    nc.tensor.matmul(wps2[:], wlhs[:], wsrc[:],
                                     start=(r == 0), stop=(r == 44))

                # w1[32:65] contribution; hpa1 = (h1acc+pb1)*e^{d1};
                # layer-1 attention interleaved per 4-jt block
                pg1 = agg1.tile([33, RPC], F32, tag="pg1")
                for q in range(NJT // 4):
                    pb1 = p1bp.tile([128, 4, 34], F32, tag="p1y")
                    for k in range(4):
                        jt = 4 * q + k
                        nc.tensor.matmul(pb1[:, k, :],
                                         cAll2[:, jt * 128:(jt + 1) * 128],
                                         w1el[:])
                    hstg = hsp.tile([128, 4, 33], F32, tag="hstg")
                    nc.vector.tensor_tensor(
                        hstg[:, :, 0:32],
                        h1acc[:, 4 * q:4 * q + 4, 0:32],
                        pb1[:, :, 0:32], op=OP.add)
                    nc.vector.tensor_tensor(
                        d1raw[:, 4 * q:4 * q + 4],
                        h1acc[:, 4 * q:4 * q + 4, 32],
                        pb1[:, :, 32], op=OP.add)
                    nc.scalar.activation(b1[:, 4 * q:4 * q + 4],
                                         d1raw[:, 4 * q:4 * q + 4],
                                         AF.Exp)
                    nc.scalar.activation(g01[:, 4 * q:4 * q + 4],
                                         d1raw[:, 4 * q:4 * q + 4],
                                         AF.Exp, scale=-0.8)
                    for k in range(4):
                        jt = 4 * q + k
                        nc.vector.tensor_scalar(
                            hpa1[:, jt, 0:32], hstg[:, k, 0:32],
                            b1[:, jt:jt + 1], None, op0=OP.mult)
                    nc.vector.tensor_copy(
                        hpa1[:, 4 * q:4 * q + 4, 32],
                        b1[:, 4 * q:4 * q + 4])
                    for k in range(4):
                        jt = 4 * q + k
                        e1 = e1pool.tile([128, RPC], BF16, tag="e1")
                        nc.vector.tensor_scalar(
                            e1[:], atile1[:],
                            g01[:, jt:jt + 1], None, op0=OP.max)
                        nc.tensor.matmul(pg1[:], hpa1[:, jt, :], e1[:],
                                         start=(jt == 0),
                                         stop=(jt == NJT - 1))

                # normalize + output
                nc.vector.tensor_copy(os1[:], pg1[:])
                for q in range(4):
                    pt2 = tp2.tile([128, 33], F32, tag="ptq33")
                    nc.tensor.matmul(pt2[:],
                                     os1[:, q * 128:(q + 1) * 128],
                                     ident[0:33, 0:33], is_transpose=True)
                    rq = otp.tile([128, 1], F32, tag="rq")
                    nc.vector.reciprocal_approx_fast(rq[:], pt2[:, 32:33])
                    ob = otp.tile([128, 32], F32, tag="ob")
                    nc.vector.tensor_scalar(ob[:], pt2[:, 0:32],
                                            rq[:, 0:1], None, op0=OP.mult)
                    nc.sync.dma_start(out_d[q * 128:(q + 1) * 128, :], ob[:])

    nc.compile()
    return nc


def _fold(inputs):
    """Host-side BN/bias/attention-projection folding (numpy, f64)."""
    f64 = np.float64
    x = np.asarray(inputs["x"], np.float32)
    xT = np.ascontiguousarray(x.T)              # [32, 4096]
    w0 = np.asarray(inputs["w0"], f64)          # [8, 32, 8]
    w1 = np.asarray(inputs["w1"], f64)          # [1, 64, 32]
    a_src0 = np.asarray(inputs["a_src0"], f64)[..., 0]   # [8, 8]
    a_dst0 = np.asarray(inputs["a_dst0"], f64)[..., 0]   # [8, 8]
    a_src1 = np.asarray(inputs["a_src1"], f64)[0, :, 0]  # [32]
    a_dst1 = np.asarray(inputs["a_dst1"], f64)[0, :, 0]  # [32]
    b0v = np.asarray(inputs["b0"], f64)         # [8]
    b1v = np.asarray(inputs["b1"], f64)         # [32]

    al0 = np.asarray(inputs["bn0_gamma"], f64) / np.sqrt(
        np.asarray(inputs["bn0_var"], f64) + BN_EPS)
    sh0 = np.asarray(inputs["bn0_beta"], f64) - \
        np.asarray(inputs["bn0_mean"], f64) * al0
    al1 = np.asarray(inputs["bn1_gamma"], f64) / np.sqrt(
        np.asarray(inputs["bn1_var"], f64) + BN_EPS)
    sh1 = np.asarray(inputs["bn1_beta"], f64) - \
        np.asarray(inputs["bn1_mean"], f64) * al1

    w0flat = (al0[None, :, None] * w0).transpose(1, 0, 2).reshape(32, 64)
    beta0h = np.einsum("i,hio->ho", sh0, w0)     # [8, 8]
    beta0 = (beta0h + b0v[None, :]).reshape(64)
    as0 = al0[:, None] * np.einsum("hio,ho->ih", w0, a_src0)   # [32, 8]
    sb0 = np.einsum("ho,ho->h", beta0h, a_src0)
    ad0 = al0[:, None] * np.einsum("hio,ho->ih", w0, a_dst0)
    db0 = np.einsum("ho,ho->h", beta0h, a_dst0)

    w0all = np.zeros((33, 72), f64)
    w0all[0:32, 0:64] = w0flat
    w0all[32, 0:64] = beta0
    w0all[0:32, 64:72] = ad0
    w0all[32, 64:72] = db0
    w0s = np.zeros((33, 8), f64)
    w0s[0:32, :] = as0
    w0s[32, :] = sb0

    # layer 1 folds; input arrives as contp = elu(out0) + 1, so the
    # -1 offset is folded into the beta/d/s rows
    w1m = w1[0]                                   # [64, 32]
    w1flat = al1[:, None] * w1m
    beta1 = sh1 @ w1m + b1v - w1flat.sum(axis=0)
    as1 = al1 * (w1m @ a_src1)
    sb1 = (sh1 @ w1m) @ a_src1 - as1.sum()
    ad1 = al1 * (w1m @ a_dst1)
    db1 = (sh1 @ w1m) @ a_dst1 - ad1.sum()

    w1e = np.zeros((65, 34), f64)
    w1e[0:64, 0:32] = w1flat
    w1e[64, 0:32] = beta1
    w1e[0:64, 32] = ad1
    w1e[64, 32] = db1
    w1e[0:64, 33] = as1
    w1e[64, 33] = sb1
    w1ea = w1e[0:32, :]
    w1el = np.concatenate([w1e[32:64, :], w1e[64:65, :]], axis=0)
    w1s1c = np.zeros((65, 1), f64)
    w1s1c[0:64, 0] = as1
    w1s1c[64, 0] = sb1

    sela = np.zeros((8, 8, 128), ml_dtypes.bfloat16)
    for h in range(8):
        sela[h, h, :] = 1.0

    xTo = np.ones((33, N), np.float32)
    xTo[0:32, :] = xT
    bf = ml_dtypes.bfloat16
    return {
        "xTb": xTo.astype(bf),
        "w0allb": w0all.astype(bf),
        "w0sb": w0s.astype(bf),
        "w1ea": w1ea.astype(bf),
        "w1el": w1el.astype(bf),
        "w1s1": w1s1c.astype(bf),
        "sela": sela.reshape(8, 8 * 128),
        "onesN": np.ones((1, N), bf),
    }


def kernel(**inputs) -> np.ndarray:
    if "nc" not in _CACHE:
        _CACHE["nc"] = _build()
    nc = _CACHE["nc"]

    shared = _fold(inputs)
    xTb = shared["xTb"]
    in_maps = []
    for c in range(NCORES):
        m = dict(shared)
        m["xsTb"] = np.ascontiguousarray(xTb[:, c * RPC:(c + 1) * RPC])
        in_maps.append(m)

    res = run_bass_kernel_spmd(nc, in_maps, list(range(NCORES)))
    out = np.concatenate([res.results[c]["out"] for c in range(NCORES)],
                         axis=0)
    return out.astype(np.float32)


# revision 63
# speedup vs baseline: 1.6103x; 1.6103x over previous
"""GAT (2-layer dense-graph attention over 4096 nodes) as a Trainium2
Bass/Tile SPMD kernel across 8 NeuronCores.

v3 design:
- Layer 0 DST-sharded (512 destination rows/core). Layer 1 ALSO
  dst-sharded: the elu'd layer-0 output (contp, [64,512] bf16/core) is
  AllGathered and every core computes its own 512 rows of layer 1
  locally in f32 PSUM (no ReduceScatter, no bf16 partial-sum loss).
  AG1 (heads 0-3) fires halfway through layer 0 and is hidden under
  pass 2; only AG2's latency is exposed (filled with AG1-dependent
  layer-1 prep matmuls, local-rank work and keep-warm matmuls).
- Layer-0 attention: 2 passes x 4 heads. Head pairs share a PSUM bank
  (odd head at tile_position col 32); the bank is DVE-memset to zero
  and every matmul uses start=False, which is correct regardless of
  stale has_written state. Inner loop is jt-major so the even/odd-head
  matmuls of a pair hit different 32-col groups back-to-back and can
  overlap in the PE sub-arrays.
- E-tile = e^{d_j} * max(a_i, e^{-0.8 d_j}) with e^{d_j} folded into
  the stationary ([e^d | h'*e^d], 9 cols, ones-free): ONE single-
  scalar DVE tensor_scalar(max) per (h,jt) tile for 3 heads/pass; the
  4th head runs on ScalarE as relu(e^{-0.8 d_j} - a_i) (per-partition
  bias AP) plus an exact rank-1 correction a_i * (sum_j e^{d_j} h'_j)
  accumulated via tiny K=1 matmuls, so DVE and ScalarE generate
  E-tiles concurrently.
- Normalize+elu is transpose-first: all per-dst math runs dst-major on
  128 lanes; one bf16 transpose per 128-dst chunk writes 32-aligned
  feature-major contp blocks.
- Output path: transpose, then per-partition reciprocal+scale.
"""

import numpy as np
import ml_dtypes

import concourse.bacc as bacc
import concourse.mybir as mybir
import concourse.tile as tile
from concourse import masks
from concourse.bass_utils import run_bass_kernel_spmd

F32 = mybir.dt.float32
BF16 = mybir.dt.bfloat16
AF = mybir.ActivationFunctionType
OP = mybir.AluOpType
N = 4096
NCORES = 8
RPC = N // NCORES          # rows per core = 512
NJT = N // 128             # 32 j-tiles of 128 source rows
BN_EPS = 1e-5

# head offloaded to ScalarE per pass (pass0: heads 0-3, pass1: 4-7);
# must be even (pair-bank col offset 0)
SCE_HEADS = (0, 4)

_CACHE = {}


def _build():
    nc = bacc.Bacc("TRN2", target_bir_lowering=False, debug=False,
                   num_devices=NCORES)

    xTb_d = nc.dram_tensor("xTb", [33, N], BF16, kind="ExternalInput")
    xsTb_d = nc.dram_tensor("xsTb", [33, RPC], BF16, kind="ExternalInput")
    w0allb_d = nc.dram_tensor("w0allb", [33, 72], BF16, kind="ExternalInput")
    w0sb_d = nc.dram_tensor("w0sb", [33, 8], BF16, kind="ExternalInput")
    w1ea_d = nc.dram_tensor("w1ea", [32, 34], BF16, kind="ExternalInput")
    w1el_d = nc.dram_tensor("w1el", [33, 34], BF16, kind="ExternalInput")
    w1s1_d = nc.dram_tensor("w1s1", [65, 1], BF16, kind="ExternalInput")
    sela_d = nc.dram_tensor("sela", [8, 8 * 128], BF16, kind="ExternalInput")
    onesN_d = nc.dram_tensor("onesN", [1, N], BF16, kind="ExternalInput")
    out_d = nc.dram_tensor("out", [RPC, 32], F32, kind="ExternalOutput")

    with tile.TileContext(nc) as tc:
        with (
            tc.tile_pool(name="const", bufs=1) as const,
            tc.tile_pool(name="per", bufs=1) as per,
            tc.tile_pool(name="dram", bufs=1, space="DRAM") as dram,
        ):
            # ---------- dram intermediates ----------
            dum_i = dram.tile([1, 8], F32, name="dum_i", tag="dum_i")
            dum_o = dram.tile([8, 8], F32, name="dum_o", tag="dum_o")
            ag1i = dram.tile([32, RPC], BF16, name="ag1i", tag="ag1i")
            ag1o = dram.tile([NCORES, 32, RPC], BF16, name="ag1o", tag="ag1o")
            ag2i = dram.tile([32, RPC], BF16, name="ag2i", tag="ag2i")
            ag2o = dram.tile([NCORES, 32, RPC], BF16, name="ag2o", tag="ag2o")

            # dummy collective first: absorbs device barrier + CC warmup
            nc.gpsimd.collective_compute(
                "AllGather", OP.bypass,
                replica_groups=[list(range(NCORES))],
                ins=[dum_i.opt()], outs=[dum_o.opt()])

            # ---------- persistent sbuf ----------
            xTb = per.tile([33, N], BF16)
            xsTb = per.tile([33, RPC], BF16)
            atile = per.tile([128, 8, RPC], BF16)      # e^{0.8 s0} bcast
            hpa0 = per.tile([128, NJT, 8, 9], BF16)    # [e^d | h'0*e^d]
            d0raw = per.tile([128, NJT * 8], F32)      # d0 scores j-major
            b0f = per.tile([128, NJT * 8], F32)        # e^{d0}
            g0 = per.tile([128, NJT * 8], F32)         # e^{-0.8 d0}
            uT = [per.tile([1, 9], BF16, name=f"uT{k}", tag=f"uT{k}")
                  for k in range(2)]
            sU = per.tile([9, 2], F32)
            contp = per.tile([65, RPC], BF16)          # elu rows + ones row
            cAll1 = per.tile([32, N], BF16)            # gathered elu 0:32
            cAll2 = per.tile([33, N], BF16)            # elu 32:64 + ones
            h1acc = per.tile([128, NJT, 34], F32)      # w1[0:32] partials
            hpa1 = per.tile([128, NJT, 33], BF16)      # [h'1*e^d1 | e^d1]
            d1raw = per.tile([128, NJT], F32)
            b1 = per.tile([128, NJT], F32)             # e^{d1}
            g01 = per.tile([128, NJT], F32)            # e^{-0.8 d1}
            a1b = per.tile([1, RPC], BF16)
            atile1 = per.tile([128, RPC], BF16)
            os1 = per.tile([33, RPC], F32)             # L1 numerator evac
            sU1 = per.tile([33, 1], F32)
            uT1 = per.tile([1, 33], BF16)

            # ---------- consts (cheap memsets first for warmup) ----------
            wsrc = const.tile([128, 512], BF16)
            nc.vector.memset(wsrc[:], 0.5)
            wlhs = const.tile([128, 128], BF16)
            nc.vector.memset(wlhs[:], 0.25)
            ones128 = const.tile([1, 128], BF16)
            nc.vector.memset(ones128[:], 1.0)
            onesc = const.tile([128, 1], BF16)
            nc.vector.memset(onesc[:], 1.0)
            ident = const.tile([128, 128], F32)
            masks.make_identity(nc, ident[:])
            nc.sync.dma_start(xsTb[:], xsTb_d[:])
            sela = const.tile([8, 8 * 128], BF16)
            nc.sync.dma_start(sela[:], sela_d[:])
            w0allb = const.tile([33, 72], BF16)
            nc.sync.dma_start(w0allb[:], w0allb_d[:])
            w0sb = const.tile([33, 8], BF16)
            nc.sync.dma_start(w0sb[:], w0sb_d[:])
            w1ea = const.tile([32, 34], BF16)
            nc.sync.dma_start(w1ea[:], w1ea_d[:])
            w1el = const.tile([33, 34], BF16)
            nc.sync.dma_start(w1el[:], w1el_d[:])
            w1s1 = const.tile([65, 1], BF16)
            nc.sync.dma_start(w1s1[:], w1s1_d[:])
            nc.sync.dma_start(contp[64:65, :], onesN_d[0:1, 0:RPC])
            nc.sync.dma_start(cAll2[32:33, :], onesN_d[:])
            for cc in range(4):
                nc.sync.dma_start(xTb[:, cc * 1024:(cc + 1) * 1024],
                                  xTb_d[:, cc * 1024:(cc + 1) * 1024])

            # ---------------- Phase 0: warmup ----------------
            with tc.tile_pool(name="wupp", bufs=1, space="PSUM") as wupp:
                wps = wupp.tile([128, RPC], F32, tag="wps")
                for r in range(10):
                    nc.tensor.matmul(wps[:], wlhs[:], wsrc[:],
                                     start=(r == 0), stop=(r == 9))

            # ---------------- Phase 1: s0 -> atile ----------------
            with (
                tc.tile_pool(name="p1s", bufs=2) as p1s,
                tc.tile_pool(name="p1p", bufs=2, space="PSUM") as p1p,
            ):
                ps0 = p1p.tile([8, RPC], F32, tag="pa")
                nc.tensor.matmul(ps0[:], w0sb[:], xsTb[:])
                a0row = p1s.tile([8, RPC], BF16, tag="a0row")
                nc.scalar.activation(a0row[:], ps0[:], AF.Exp, scale=0.8)
                for h in range(8):
                    pa = p1p.tile([128, RPC], F32, tag="pa")
                    nc.tensor.matmul(pa[:], sela[:, h * 128:(h + 1) * 128],
                                     a0row[:])
                    if h % 2 == 0:
                        nc.vector.tensor_copy(atile[:, h, :], pa[:])
                    else:
                        nc.scalar.copy(atile[:, h, :], pa[:])

            # -------- Layer-0 attention: two passes of 4 heads --------
            def prep_group(g):
                """h'0/d0 for j-tiles 8g..8g+8 -> hpa0 (e^d folded)."""
                gs = slice(g * 64, (g + 1) * 64)
                gj = slice(g * 8, (g + 1) * 8)
                pts = []
                for half in range(2):
                    pt = ptp.tile([128, 4, 72], F32, tag="ptile")
                    pts.append(pt)
                    for q in range(4):
                        jt = g * 8 + half * 4 + q
                        nc.tensor.matmul(
                            pt[:, q, :],
                            xTb[:, jt * 128:(jt + 1) * 128],
                            w0allb[:])
                        nc.vector.tensor_copy(
                            d0raw[:, jt * 8:(jt + 1) * 8],
                            pt[:, q, 64:72])
                nc.scalar.activation(g0[:, gs], d0raw[:, gs], AF.Exp,
                                     scale=-0.8)
                nc.scalar.activation(b0f[:, gs], d0raw[:, gs], AF.Exp)
                nc.scalar.activation(
                    hpa0[:, gj, :, 0], d0raw[:, gs].rearrange(
                        "p (a b) -> p a b", b=8), AF.Exp)
                for half in range(2):
                    for q in range(4):
                        jt = g * 8 + half * 4 + q
                        bc = b0f[:, jt * 8:(jt + 1) * 8]
                        nc.vector.tensor_tensor(
                            hpa0[:, jt, :, 1:9],
                            pts[half][:, q, 0:64].rearrange(
                                "p (h o) -> p h o", h=8),
                            bc.unsqueeze(2).broadcast_to([128, 8, 8]),
                            op=OP.mult)

            def run_pass(p, pgp, psu, prep):
                heads = list(range(4 * p, 4 * p + 4))
                sce_h = SCE_HEADS[p]
                psU = psu.tile([9, 1], F32, tag="psU")
                pgs = {}
                for h in heads:
                    # one PSUM bank per head: adjacent matmuls alternate
                    # 32-col groups (concurrency) but never share a bank
                    pgs[h] = pgp.tile([41, RPC], F32,
                                      name=f"pg_{p}_{h}",
                                      tag=f"pg{h % 4}")
                    if h % 2 == 0:
                        nc.vector.memset(pgs[h][0:9, :], 0.0)
                    else:
                        nc.vector.memset(pgs[h][32:41, :], 0.0)
                if prep:  # stay one group ahead so ScE exps clear the
                    prep_group(0)  # queue before the group's relu tiles
                    prep_group(1)
                for g in range(4):
                    if prep and 2 <= g + 1 <= 3:
                        prep_group(g + 1)
                    for j8 in range(8):
                        jt = g * 8 + j8
                        es = {}
                        for h in heads:
                            col = jt * 8 + h
                            e = epool.tile([128, RPC], BF16, tag="e")
                            es[h] = e
                            if h == sce_h:
                                nc.scalar.activation(
                                    e[:], atile[:, h, :], AF.Relu,
                                    bias=g0[:, col:col + 1],
                                    scale=-1.0)
                            else:
                                nc.vector.tensor_scalar(
                                    e[:], atile[:, h, :],
                                    g0[:, col:col + 1], None,
                                    op0=OP.max)
                        for h in heads:
                            hv = pgs[h][0:9, :] if h % 2 == 0 \
                                else pgs[h][32:41, :]
                            tp = (0, 0) if h % 2 == 0 else (0, 32)
                            nc.tensor.matmul(
                                hv, hpa0[:, jt, h, :], es[h][:],
                                start=False,
                                stop=(jt == NJT - 1 and h != sce_h),
                                skip_group_check=True,
                                tile_position=tp)
                            if h == sce_h:
                                nc.tensor.matmul(
                                    psU[:], hpa0[:, jt, h, :],
                                    onesc[:],
                                    start=(jt == 0),
                                    stop=(jt == NJT - 1))
                # rank-1 correction for the ScalarE head (even => col 0)
                nc.vector.tensor_copy(sU[:, p:p + 1], psU[:])
                put = psu.tile([1, 9], F32, tag="psUT")
                nc.tensor.matmul(put[:], sU[:, p:p + 1], ident[0:9, 0:9],
                                 is_transpose=True)
                nc.vector.tensor_copy(uT[p][:], put[:])
                nc.tensor.matmul(pgs[sce_h][0:9, :], uT[p][:],
                                 atile[0:1, sce_h, :],
                                 start=False, stop=True,
                                 skip_group_check=True)
                return pgs

            def finalize_pass(p, pgs, agi, finp, sbfp):
                """normalize+elu for the pass's 4 heads -> contp rows
                32p:32p+32, transpose-first (all dst-major on 128
                lanes), then bf16 transpose back per 128-dst chunk."""
                sbA = sbfp.tile([41, RPC], F32, tag="sbf")
                nc.vector.tensor_copy(sbA[0:9, :], pgs[4 * p][0:9, :])
                nc.scalar.copy(sbA[32:41, :], pgs[4 * p + 1][32:41, :])
                sbB = sbfp.tile([41, RPC], F32, tag="sbf")
                nc.vector.tensor_copy(sbB[0:9, :], pgs[4 * p + 2][0:9, :])
                nc.scalar.copy(sbB[32:41, :], pgs[4 * p + 3][32:41, :])
                for q in range(4):
                    nm0 = sbfp.tile([128, 32], F32, tag="nm0")
                    for c2, sb in ((0, sbA), (1, sbB)):
                        ptx = finp.tile([128, 41], F32, tag="ptx")
                        nc.tensor.matmul(ptx[:],
                                         sb[:, q * 128:(q + 1) * 128],
                                         ident[0:41, 0:41],
                                         is_transpose=True)
                        rq2 = sbfp.tile([128, 2], F32, tag="rq2")
                        nc.vector.reciprocal_approx_fast(
                            rq2[:, 0:1], ptx[:, 0:1])
                        nc.vector.reciprocal_approx_fast(
                            rq2[:, 1:2], ptx[:, 32:33])
                        nc.vector.tensor_scalar(
                            nm0[:, 16 * c2:16 * c2 + 8], ptx[:, 1:9],
                            rq2[:, 0:1], None, op0=OP.mult)
                        nc.vector.tensor_scalar(
                            nm0[:, 16 * c2 + 8:16 * c2 + 16],
                            ptx[:, 33:41],
                            rq2[:, 1:2], None, op0=OP.mult)
                    # contp carries elu+1 = max(x,0) + min(e^x,1); the
                    # -1 is folded into the layer-1 beta/d/s rows
                    texq = sbfp.tile([128, 32], F32, tag="tex")
                    nc.scalar.activation(texq[:], nm0[:], AF.Exp)
                    t1q = sbfp.tile([128, 32], F32, tag="tex")
                    nc.vector.tensor_scalar(t1q[:], texq[:], 1.0, None,
                                            op0=OP.min)
                    eluq = sbfp.tile([128, 32], F32, tag="eluq")
                    nc.vector.scalar_tensor_tensor(
                        eluq[:], nm0[:], 0.0, t1q[:],
                        op0=OP.max, op1=OP.add)
                    ptb = finp.tile([32, 128], F32, tag="ptb")
                    nc.tensor.matmul(ptb[:], eluq[:],
                                     ident[0:128, 0:128],
                                     is_transpose=True)
                    nc.vector.tensor_copy(
                        contp[32 * p:32 * p + 32,
                              q * 128:(q + 1) * 128], ptb[:])
                nc.sync.dma_start(agi[:],
                                  contp[32 * p:32 * p + 32, :])

            with (
                tc.tile_pool(name="epool", bufs=8) as epool,
                tc.tile_pool(name="sbfp", bufs=2) as sbfp,
            ):
                with tc.tile_pool(name="pgp", bufs=1, space="PSUM") as pgp:
                    with (
                        tc.tile_pool(name="ptp", bufs=2,
                                     space="PSUM") as ptp,
                        tc.tile_pool(name="psu", bufs=1,
                                     space="PSUM") as psu,
                    ):
                        pgs1 = run_pass(0, pgp, psu, prep=True)
                    with (
                        tc.tile_pool(name="fin", bufs=1,
                                     space="PSUM") as finp,
                        tc.tile_pool(name="psu2", bufs=1,
                                     space="PSUM") as psu2,
                    ):
                        finalize_pass(0, pgs1, ag1i, finp, sbfp)
                        nc.gpsimd.collective_compute(
                            "AllGather", OP.bypass,
                            replica_groups=[list(range(NCORES))],
                            ins=[ag1i.opt()], outs=[ag1o.opt()])
                        for r in range(NCORES):
                            nc.sync.dma_start(
                                cAll1[:, r * RPC:(r + 1) * RPC],
                                ag1o[r, :, :])
                        pgs2 = run_pass(1, pgp, psu2, prep=False)
                    with (
                        tc.tile_pool(name="fin2", bufs=1,
                                     space="PSUM") as finp2,
                        tc.tile_pool(name="p1ap", bufs=1,
                                     space="PSUM") as p1ap,
                        tc.tile_pool(name="tp2a", bufs=1,
                                     space="PSUM") as tp2a,
                    ):
                        # AG1-dependent layer-1 prep (runs while pass-2
                        # finalize uses DVE/ScE): w1[0:32] contribution
                        for q in range(NJT // 4):
                            pa1 = p1ap.tile([128, 4, 34], F32, tag="p1x")
                            for k in range(4):
                                jt = 4 * q + k
                                nc.tensor.matmul(
                                    pa1[:, k, :],
                                    cAll1[:, jt * 128:(jt + 1) * 128],
                                    w1ea[:])
                            nc.vector.tensor_copy(
                                h1acc[:, 4 * q:4 * q + 4, :], pa1[:])
                        finalize_pass(1, pgs2, ag2i, finp2, sbfp)
                        # own-rank L1 pieces + a1 broadcast (local data;
                        # must come AFTER finalize writes contp 32:64)
                        ps1r = tp2a.tile([128, RPC], F32, tag="pbig")
                        nc.tensor.matmul(ps1r[0:1, :], w1s1[:], contp[:])
                        nc.scalar.activation(a1b[:], ps1r[0:1, :], AF.Exp,
                                             scale=0.8)
                        pa1b = tp2a.tile([128, RPC], F32, tag="pbig")
                        nc.tensor.matmul(pa1b[:], ones128[:], a1b[:])
                        nc.vector.tensor_copy(atile1[:], pa1b[:])

            nc.gpsimd.collective_compute(
                "AllGather", OP.bypass,
                replica_groups=[list(range(NCORES))],
                ins=[ag2i.opt()], outs=[ag2o.opt()])

            # ---------------- Layer 1 ----------------
            with (
                tc.tile_pool(name="p1bp", bufs=2, space="PSUM") as p1bp,
                tc.tile_pool(name="agg1", bufs=1, space="PSUM") as agg1,
                tc.tile_pool(name="psu3", bufs=1, space="PSUM") as psu3,
                tc.tile_pool(name="tp2", bufs=1, space="PSUM") as tp2,
                tc.tile_pool(name="hsp", bufs=2) as hsp,
                tc.tile_pool(name="e1pool", bufs=4) as e1pool,
                tc.tile_pool(name="otp", bufs=2) as otp,
            ):
                for r in range(NCORES):
                    nc.sync.dma_start(cAll2[0:32, r * RPC:(r + 1) * RPC],
                                      ag2o[r, :, :])
                # keep PE warm across the AG2 wait (also spans the gap
                # so the layer-1 stream starts at full clock)
                wps2 = tp2.tile([128, RPC], F32, tag="pbig")
                for r in range(45):
                    nc.tensor.matmul(wps2[:], wlhs[:], wsrc[:],
                                     start=(r == 0), stop=(r == 44))

                # w1[32:65] contribution; hpa1 = (h1acc+pb1)*e^{d1};
                # layer-1 attention interleaved per 4-jt block
                pg1 = agg1.tile([33, RPC], F32, tag="pg1")
                psU1 = psu3.tile([33, 1], F32, tag="psU1")
                for q in range(NJT // 4):
                    pb1 = p1bp.tile([128, 4, 34], F32, tag="p1y")
                    for k in range(4):
                        jt = 4 * q + k
                        nc.tensor.matmul(pb1[:, k, :],
                                         cAll2[:, jt * 128:(jt + 1) * 128],
                                         w1el[:])
                    hstg = hsp.tile([128, 4, 33], F32, tag="hstg")
                    nc.vector.tensor_tensor(
                        hstg[:, :, 0:32],
                        h1acc[:, 4 * q:4 * q + 4, 0:32],
                        pb1[:, :, 0:32], op=OP.add)
                    nc.vector.tensor_tensor(
                        d1raw[:, 4 * q:4 * q + 4],
                        h1acc[:, 4 * q:4 * q + 4, 32],
                        pb1[:, :, 32], op=OP.add)
                    nc.scalar.activation(b1[:, 4 * q:4 * q + 4],
                                         d1raw[:, 4 * q:4 * q + 4],
                                         AF.Exp)
                    nc.scalar.activation(g01[:, 4 * q:4 * q + 4],
                                         d1raw[:, 4 * q:4 * q + 4],
                                         AF.Exp, scale=-0.8)
                    for k in range(4):
                        jt = 4 * q + k
                        nc.vector.tensor_scalar(
                            hpa1[:, jt, 0:32], hstg[:, k, 0:32],
                            b1[:, jt:jt + 1], None, op0=OP.mult)
                    nc.vector.tensor_copy(
                        hpa1[:, 4 * q:4 * q + 4, 32],
                        b1[:, 4 * q:4 * q + 4])
                    for k in range(4):
                        jt = 4 * q + k
                        e1 = e1pool.tile([128, RPC], BF16, tag="e1")
                        if jt % 3 == 2:
                            # relu-form on ScalarE; rank-1 corrected
                            nc.scalar.activation(
                                e1[:], atile1[:], AF.Relu,
                                bias=g01[:, jt:jt + 1], scale=-1.0)
                            nc.tensor.matmul(
                                psU1[:], hpa1[:, jt, :], onesc[:],
                                start=(jt == 2), stop=(jt == 29))
                        else:
                            nc.vector.tensor_scalar(
                                e1[:], atile1[:],
                                g01[:, jt:jt + 1], None, op0=OP.max)
                        nc.tensor.matmul(pg1[:], hpa1[:, jt, :], e1[:],
                                         start=(jt == 0),
                                         stop=False,
                                         skip_group_check=True)
                # rank-1 correction for the ScalarE-assigned j-tiles
                nc.vector.tensor_copy(sU1[:], psU1[:])
                put1 = psu3.tile([1, 33], F32, tag="put1")
                nc.tensor.matmul(put1[:], sU1[:], ident[0:33, 0:33],
                                 is_transpose=True)
                nc.vector.tensor_copy(uT1[:], put1[:])
                nc.tensor.matmul(pg1[:], uT1[:], atile1[0:1, :],
                                 start=False, stop=True,
                                 skip_group_check=True)

                # normalize + output
                nc.vector.tensor_copy(os1[:], pg1[:])
                for q in range(4):
                    pt2 = tp2.tile([128, 33], F32, tag="ptq33")
                    nc.tensor.matmul(pt2[:],
                                     os1[:, q * 128:(q + 1) * 128],
                                     ident[0:33, 0:33], is_transpose=True)
                    rq = otp.tile([128, 1], F32, tag="rq")
                    nc.vector.reciprocal_approx_fast(rq[:], pt2[:, 32:33])
                    ob = otp.tile([128, 32], F32, tag="ob")
                    nc.vector.tensor_scalar(ob[:], pt2[:, 0:32],
                                            rq[:, 0:1], None, op0=OP.mult)
                    nc.sync.dma_start(out_d[q * 128:(q + 1) * 128, :], ob[:])

    nc.compile()
    return nc


def _fold(inputs):
    """Host-side BN/bias/attention-projection folding (numpy, f64)."""
    f64 = np.float64
    x = np.asarray(inputs["x"], np.float32)
    xT = np.ascontiguousarray(x.T)              # [32, 4096]
    w0 = np.asarray(inputs["w0"], f64)          # [8, 32, 8]
    w1 = np.asarray(inputs["w1"], f64)          # [1, 64, 32]
    a_src0 = np.asarray(inputs["a_src0"], f64)[..., 0]   # [8, 8]
    a_dst0 = np.asarray(inputs["a_dst0"], f64)[..., 0]   # [8, 8]
    a_src1 = np.asarray(inputs["a_src1"], f64)[0, :, 0]  # [32]
    a_dst1 = np.asarray(inputs["a_dst1"], f64)[0, :, 0]  # [32]
    b0v = np.asarray(inputs["b0"], f64)         # [8]
    b1v = np.asarray(inputs["b1"], f64)         # [32]

    al0 = np.asarray(inputs["bn0_gamma"], f64) / np.sqrt(
        np.asarray(inputs["bn0_var"], f64) + BN_EPS)
    sh0 = np.asarray(inputs["bn0_beta"], f64) - \
        np.asarray(inputs["bn0_mean"], f64) * al0
    al1 = np.asarray(inputs["bn1_gamma"], f64) / np.sqrt(
        np.asarray(inputs["bn1_var"], f64) + BN_EPS)
    sh1 = np.asarray(inputs["bn1_beta"], f64) - \
        np.asarray(inputs["bn1_mean"], f64) * al1

    w0flat = (al0[None, :, None] * w0).transpose(1, 0, 2).reshape(32, 64)
    beta0h = np.einsum("i,hio->ho", sh0, w0)     # [8, 8]
    beta0 = (beta0h + b0v[None, :]).reshape(64)
    as0 = al0[:, None] * np.einsum("hio,ho->ih", w0, a_src0)   # [32, 8]
    sb0 = np.einsum("ho,ho->h", beta0h, a_src0)
    ad0 = al0[:, None] * np.einsum("hio,ho->ih", w0, a_dst0)
    db0 = np.einsum("ho,ho->h", beta0h, a_dst0)

    w0all = np.zeros((33, 72), f64)
    w0all[0:32, 0:64] = w0flat
    w0all[32, 0:64] = beta0
    w0all[0:32, 64:72] = ad0
    w0all[32, 64:72] = db0
    w0s = np.zeros((33, 8), f64)
    w0s[0:32, :] = as0
    w0s[32, :] = sb0

    # layer 1 folds; input arrives as contp = elu(out0) + 1, so the
    # -1 offset is folded into the beta/d/s rows
    w1m = w1[0]                                   # [64, 32]
    w1flat = al1[:, None] * w1m
    beta1 = sh1 @ w1m + b1v - w1flat.sum(axis=0)
    as1 = al1 * (w1m @ a_src1)
    sb1 = (sh1 @ w1m) @ a_src1 - as1.sum()
    ad1 = al1 * (w1m @ a_dst1)
    db1 = (sh1 @ w1m) @ a_dst1 - ad1.sum()

    w1e = np.zeros((65, 34), f64)
    w1e[0:64, 0:32] = w1flat
    w1e[64, 0:32] = beta1
    w1e[0:64, 32] = ad1
    w1e[64, 32] = db1
    w1e[0:64, 33] = as1
    w1e[64, 33] = sb1
    w1ea = w1e[0:32, :]
    w1el = np.concatenate([w1e[32:64, :], w1e[64:65, :]], axis=0)
    w1s1c = np.zeros((65, 1), f64)
    w1s1c[0:64, 0] = as1
    w1s1c[64, 0] = sb1

    sela = np.zeros((8, 8, 128), ml_dtypes.bfloat16)
    for h in range(8):
        sela[h, h, :] = 1.0

    xTo = np.ones((33, N), np.float32)
    xTo[0:32, :] = xT
    bf = ml_dtypes.bfloat16
    return {
        "xTb": xTo.astype(bf),
        "w0allb": w0all.astype(bf),
        "w0sb": w0s.astype(bf),
        "w1ea": w1ea.astype(bf),
        "w1el": w1el.astype(bf),
        "w1s1": w1s1c.astype(bf),
        "sela": sela.reshape(8, 8 * 128),
        "onesN": np.ones((1, N), bf),
    }


def kernel(**inputs) -> np.ndarray:
    if "nc" not in _CACHE:
        _CACHE["nc"] = _build()
    nc = _CACHE["nc"]

    shared = _fold(inputs)
    xTb = shared["xTb"]
    in_maps = []
    for c in range(NCORES):
        m = dict(shared)
        m["xsTb"] = np.ascontiguousarray(xTb[:, c * RPC:(c + 1) * RPC])
        in_maps.append(m)

    res = run_bass_kernel_spmd(nc, in_maps, list(range(NCORES)))
    out = np.concatenate([res.results[c]["out"] for c in range(NCORES)],
                         axis=0)
    return out.astype(np.float32)
